# revision 24
# baseline (speedup 1.0000x reference)
"""CrossAttention kernel for Trainium2, 8 NeuronCores.

Reference pipeline (B=4, C=256, H=W=64, N=4096, d=C//8=32):
  sub = x1 - x2
  x3 = relu(bn1(pw1(dw1([sub, x1]))))      # dw: 3x3 grouped conv (groups=C)
  x4 = relu(bn2(pw2(dw2([sub, x2]))))      # pw: 1x1 512->256
  q = wq@x4 [B,32,N]; k = wk@x3 [B,32,N]; v = wv@x3 [B,256,N]
  attn = softmax(q^T k);  out = gamma * (v @ attn^T) + x1

Sharding: 8 cores = (batch b) x (pixel-half h). Each core computes BOTH
conv paths only for its own pixel half (with a one-row halo), projects
k / v^T / q from its half, then the pair exchanges k and v^T via a
single AllGather so each core runs flash attention for its 2048 queries
over all 4096 keys.

Device-side choices:
  - dw conv on the PE as 9 accumulating block-diagonal [128x128] matmuls
    over a zero-padded 66-col image layout (8-row windows = 512 output
    columns via a strided rhs AP that skips the pad columns).
  - energy is computed transposed, E^T[j, i], with k/q zero-padded to a
    128-deep contraction so the PE streams at full rate.
  - attention accumulation outputs [channel, query] directly (lhsT =
    v^T block, rhs = exp tile): 512-wide matmuls, no PE transposes. The
    softmax denominator comes from an all-ones lhsT matmul which also
    broadcasts it across partitions for the DVE normalize.
  - all inputs packed into few large DMAs (descriptor generation on the
    sync engine costs ~0.7us per dma_start, so count matters).
  - matmul datapath in bf16 (fp32 PSUM accumulation); normalize /
    residual-add in fp32. gamma folded into wv/bv on the host; bn+biases
    folded into per-channel scale/shift applied during PSUM eviction.
"""

import numpy as np
import ml_dtypes

import concourse.bass as bass
import concourse.mybir as mybir
import concourse.tile as tile
from concourse import bacc
from concourse.bass_utils import run_bass_kernel_spmd

F32 = mybir.dt.float32
BF16 = mybir.dt.bfloat16
F8 = mybir.dt.float8e4
U8 = mybir.dt.uint8
PM = mybir.MatmulPerfMode
AF = mybir.ActivationFunctionType
ALU = mybir.AluOpType

B, C, H, W = 4, 256, 64, 64
N = H * W            # 4096 pixels
QH = N // 2          # pixels per core (queries/own keys)
EPS = 1e-5
PW = 66              # padded row width
OFF = 2              # leading pad elements in padded tiles
SLOTS = 34           # 32 data rows + halo/zero rows
CAT_F = OFF + SLOTS * PW + OFF   # 2248 (legacy bf16 layout, unused)
PW8 = 80                         # fp8 padded row stride (16B-aligned)
CAT80 = SLOTS * PW8              # 2720 flat image bytes
AREG = 2736                      # per-copy region (16B-aligned)
CATW = 2 * AREG + 9 * 128        # image copy A | shifted copy B | dw weights
VT = 272             # v^T row stride: 256 channels + pad (16B-aligned for DoubleRow)
PAIRS = [[0, 1], [2, 3], [4, 5], [6, 7]]

_CACHE = {}


def _build_nc():
    nc = bacc.Bacc("TRN2", target_bir_lowering=False, debug=False, num_devices=8)

    c1w_d = nc.dram_tensor("c1w", [4, 128, CATW], F8, kind="ExternalInput")
    c2w_d = nc.dram_tensor("c2w", [4, 128, CATW], F8, kind="ExternalInput")
    x1h_d = nc.dram_tensor("x1h", [128, 2 * QH], F32, kind="ExternalInput")
    consts_d = nc.dram_tensor("consts", [128, 11], F32, kind="ExternalInput")
    projw_d = nc.dram_tensor("projw", [128, 640], F8, kind="ExternalInput")
    pwall_d = nc.dram_tensor("pwall", [128, 2048], F8, kind="ExternalInput")
    out_d = nc.dram_tensor("out", [128, 2 * QH], F32, kind="ExternalOutput")

    # collective bounce buffers, in bytes: v^T fp8 (128x4128) + k bf16 (32x2048)
    VSZ = 128 * 16 * VT              # v^T bytes
    KVN = VSZ + 32 * QH * 2          # + k bytes = 659456
    kvown_d = nc.dram_tensor("kvown_b", [KVN], U8)
    kvfull_d = nc.dram_tensor("kvfull_b", [2 * KVN], U8)

    with tile.TileContext(nc) as tc:
        with tc.tile_pool(name="persist", bufs=1) as pp:
            catw = [[pp.tile([128, CATW], F8, name=f"cw{b}_{k}",
                             tag=f"cw{b}_{k}") for k in range(4)]
                    for b in range(2)]
            x3o = pp.tile([128, 2 * QH], F8, name="x3o", tag="x3o")
            x4 = pp.tile([128, 2 * QH], F8, name="x4", tag="x4")
            consts = pp.tile([128, 11], F32, name="consts", tag="consts")
            projw = pp.tile([128, 640], F8, name="projw", tag="projw")
            pwall = pp.tile([128, 2048], F8, name="pwall", tag="pwall")
            k_own = pp.tile([128, QH], BF16, name="k_own", tag="k_own")
            vto = pp.tile([128, 16 * VT], F8, name="vto", tag="vto")
            k_sb = pp.tile([128, N], BF16, name="k_sb", tag="k_sb")
            q_sb = pp.tile([128, QH], BF16, name="q_sb", tag="q_sb")
            vta = pp.tile([128, 32 * VT], F8, name="vta", tag="vta")
            x1h = pp.tile([128, 2 * QH], F32, name="x1h", tag="x1h")
            out_sb = pp.tile([128, 2 * QH], F32, name="osb", tag="osb")
            ones_sb = pp.tile([128, 256], F8, name="ones", tag="ones")
            # zero rows 32:128 so energy matmuls can use full 128-row lhsT
            # (avoids the PE small-tile row-group slowdown)
            nc.gpsimd.memset(k_sb[:], 0.0)
            nc.gpsimd.memset(q_sb[:], 0.0)
            nc.gpsimd.memset(ones_sb[:], 1.0)

            # input DMAs in priority order (conv1, consts, conv2, residual)
            for k in range(4):
                nc.sync.dma_start(catw[0][k][:], c1w_d[k])
            nc.sync.dma_start(consts[:], consts_d[:])
            nc.sync.dma_start(pwall[:], pwall_d[:])
            nc.sync.dma_start(projw[:], projw_d[:])
            for k in range(4):
                nc.sync.dma_start(catw[1][k][:], c2w_d[k])
            nc.sync.dma_start(x1h[:], x1h_d[:])

            pwr = pwall[:].rearrange("p (b k m) -> p b k m", b=2, k=4)

            def conv_block(cb, cw, bno, xout, after_window=None):
                # dw in bf16 (shifted-window APs rule out DoubleRow); y1
                # evicted x16 into fp8 so the 1x1 pw conv runs as fp8
                # DoubleRow k-tile pairs (weights x64; bn scale /1024)
                WO = 2 * AREG
                with tc.tile_pool(name="conv_y", bufs=2) as cyb, \
                     tc.tile_pool(name="conv_ps", bufs=2, space="PSUM") as cps:
                    for w in range(4):
                        y1w = cyb.tile([128, 2048], F8, name="y1w", tag="y1w")
                        y1r = y1w[:].rearrange("p (k f) -> p k f", k=4)
                        for k in range(4):
                            ab = cw[k][:, 0:WO].rearrange("p (c f) -> p c f",
                                                          c=2)
                            for sub in range(2):
                                w4 = 2 * w + sub
                                ps = cps.tile([128, 320], F32, name="dwps",
                                              tag="dwps")
                                for dr in range(3):
                                    s = (4 * w4 + dr) * PW8
                                    nc.tensor.matmul(
                                        ps[:],
                                        cw[k][:, WO + 384 * dr:
                                              WO + 384 * dr + 256]
                                        .rearrange("p (c f) -> p c f", c=2),
                                        ab[:, :, s:s + 320],
                                        start=(dr == 0), stop=False,
                                        perf_mode=PM.DoubleRow)
                                    nc.tensor.matmul(
                                        ps[:],
                                        cw[k][:, WO + 384 * dr + 256:
                                              WO + 384 * dr + 384],
                                        cw[k][:, s + 2:s + 322],
                                        start=False, stop=(dr == 2))
                                nc.scalar.activation(
                                    y1w[:, 512 * k + 256 * sub:
                                        512 * k + 256 * (sub + 1)],
                                    ps[:, 0:320].rearrange(
                                        "p (r c) -> p r c", r=4,
                                        c=PW8)[:, :, 0:64],
                                    AF.Copy, scale=0.25)
                        for m in range(2):
                            pp2 = cps.tile([128, 512], F32, name="pwps", tag="pwps")
                            for kp in range(2):
                                nc.tensor.matmul(
                                    pp2[:],
                                    pwr[:, cb, 2 * kp:2 * kp + 2,
                                        128 * m:128 * (m + 1)],
                                    y1r[:, 2 * kp:2 * kp + 2, :],
                                    start=(kp == 0), stop=(kp == 1),
                                    perf_mode=PM.DoubleRow)
                            nc.scalar.activation(
                                xout[:, QH * m + 512 * w:QH * m + 512 * (w + 1)],
                                pp2[:], AF.Relu,
                                bias=consts[:, bno + 2 * m + 1:bno + 2 * m + 2],
                                scale=consts[:, bno + 2 * m:bno + 2 * m + 1])
                        if after_window is not None:
                            after_window(w)

            # projections interleaved into conv1's window loop: k / v^T
            # blocks only need that window's x3 columns, so the pair
            # exchange can trigger right at conv1's end
            pps_c = tc.tile_pool(name="proj_ps", bufs=2, space="PSUM")
            pps = pps_c.__enter__()

            pjr = projw[:].rearrange("p (c f) -> p c f", c=2)
            x3r = x3o[:].rearrange("p (c f) -> p c f", c=2)
            x4r = x4[:].rearrange("p (c f) -> p c f", c=2)

            def proj1_window(s):
                ps = pps.tile([128, 512], F32, name="kqps", tag="kqps")
                nc.tensor.matmul(ps[0:32, :], pjr[:, :, 256:288],
                                 x3r[:, :, 512 * s:512 * (s + 1)],
                                 start=True, stop=True,
                                 perf_mode=PM.DoubleRow)
                nc.scalar.activation(k_own[0:32, 512 * s:512 * (s + 1)],
                                     ps[0:32, :], AF.Identity,
                                     bias=consts[0:32, 8:9])
                for j in range(4 * s, 4 * s + 4):
                    ps = pps.tile([128, 256], F32, name="vtps", tag="vtps")
                    nc.tensor.matmul(ps[:],
                                     x3r[:, :, 128 * j:128 * (j + 1)],
                                     pjr[:, :, 0:256],
                                     start=True, stop=True,
                                     perf_mode=PM.DoubleRow)
                    nc.scalar.activation(vto[:, VT * j:VT * j + 256], ps[:],
                                         AF.Copy)

            conv_block(0, catw[0], 0, x3o, after_window=proj1_window)

            if True:
                # ship own k / v^T, single AllGather for the pair
                nc.sync.dma_start(
                    kvown_d[0:VSZ].rearrange("(p f) -> p f", p=128),
                    vto[:].bitcast(U8))
                nc.sync.dma_start(
                    kvown_d[VSZ:KVN].rearrange("(p f) -> p f", p=32),
                    k_own[0:32, :].bitcast(U8))
                nc.gpsimd.collective_compute(
                    "AllGather", ALU.bypass, replica_groups=PAIRS,
                    ins=[kvown_d[:].opt()], outs=[kvfull_d[:].opt()])
                for m in range(2):
                    o = m * KVN
                    nc.sync.dma_start(
                        vta[:, 16 * VT * m:16 * VT * (m + 1)].bitcast(U8),
                        kvfull_d[o:o + VSZ].rearrange("(p f) -> p f", p=128))
                    nc.sync.dma_start(
                        k_sb[0:32, QH * m:QH * (m + 1)].bitcast(U8),
                        kvfull_d[o + VSZ:o + KVN].rearrange("(p f) -> p f",
                                                            p=32))

                # conv2 + q overlap with the collective
                def proj2_window(s):
                    ps = pps.tile([128, 512], F32, name="kqps", tag="kqps")
                    nc.tensor.matmul(ps[0:32, :], pjr[:, :, 288:320],
                                     x4r[:, :, 512 * s:512 * (s + 1)],
                                     start=True, stop=True,
                                     perf_mode=PM.DoubleRow)
                    nc.scalar.activation(q_sb[0:32, 512 * s:512 * (s + 1)],
                                         ps[0:32, :], AF.Identity,
                                         bias=consts[32:64, 8:9])

                conv_block(1, catw[1], 4, x4, after_window=proj2_window)
            pps_c.__exit__(None, None, None)

            # ---- flash attention (output computed pre-transposed) ----
            # single flattened pipeline over 64 key-block pairs (4 query
            # blocks x 16) so the PE/scalar pipeline never drains at query
            # block boundaries; normalize uses a fused divide stt so acc
            # banks free quickly
            with tc.tile_pool(name="att_sb", bufs=2) as asb, \
                 tc.tile_pool(name="acc_ps", bufs=1, space="PSUM") as accp, \
                 tc.tile_pool(name="dps_ps", bufs=2, space="PSUM") as dpsp, \
                 tc.tile_pool(name="e_ps", bufs=2, space="PSUM") as epsp:
                vv = vta[:].rearrange("p (j v) -> p j v", j=32)
                ones2 = ones_sb[:].rearrange("p (k f) -> p k f", k=2)
                eps_t = {}
                acc = None
                dps = None

                def energy_pair(g):
                    ib = g // 16
                    p = g % 16
                    eps_t[g] = epsp.tile([128, 1024], F32, name="eps",
                                         tag="eps")
                    for h in range(2):
                        nc.tensor.matmul(
                            eps_t[g][:, 512 * h:512 * (h + 1)],
                            k_sb[:, 128 * (2 * p + h):128 * (2 * p + h + 1)],
                            q_sb[:, 512 * ib:512 * (ib + 1)],
                            start=True, stop=True)

                for g in range(64):
                    ib, p = g // 16, g % 16
                    if g == 0:
                        energy_pair(0)
                    if p == 0:
                        acc = [accp.tile([128, 512], F32, name=f"acc{c}",
                                         tag=f"acc{c}") for c in range(2)]
                        dps = dpsp.tile([128, 512], F32, name="dps", tag="dps")
                    expair = asb.tile([128, 1024], F8, name="ex", tag="ex")
                    nc.scalar.activation(expair[:], eps_t[g][:], AF.Exp,
                                         scale=1.0 / 4096.0)
                    if g + 1 < 64:
                        energy_pair(g + 1)
                    eps_t.pop(g - 1, None)
                    rhs2 = expair[:].rearrange("p (k f) -> p k f", k=2)
                    nc.tensor.matmul(acc[0][:], vv[:, 2 * p:2 * p + 2, 0:128],
                                     rhs2, start=(p == 0), stop=(p == 15),
                                     perf_mode=PM.DoubleRow)
                    nc.tensor.matmul(acc[1][:],
                                     vv[:, 2 * p:2 * p + 2, 128:256],
                                     rhs2, start=(p == 0), stop=(p == 15),
                                     perf_mode=PM.DoubleRow)
                    nc.tensor.matmul(dps[:], ones2, rhs2,
                                     start=(p == 0), stop=(p == 15),
                                     perf_mode=PM.DoubleRow)
                    if p == 15:
                        # normalize: PSUM-reading ops first so acc banks
                        # free before the next query block needs them
                        rec = asb.tile([128, 512], F32, name="rec", tag="rec")
                        nc.vector.reciprocal_approx_fast(rec[:], dps[:])
                        tmp = [asb.tile([128, 512], BF16, name=f"tmp{c}",
                                        tag=f"tmp{c}") for c in range(2)]
                        for ch in range(2):
                            nc.vector.scalar_tensor_tensor(
                                tmp[ch][:], acc[ch][:], 1.0 / 64.0, rec[:],
                                ALU.mult, ALU.mult)
                        for ch in range(2):
                            nc.vector.scalar_tensor_tensor(
                                out_sb[:,
                                       QH * ch + 512 * ib:QH * ch + 512 * (ib + 1)],
                                tmp[ch][:], consts[:, 9 + ch:10 + ch],
                                x1h[:,
                                    QH * ch + 512 * ib:QH * ch + 512 * (ib + 1)],
                                ALU.add, ALU.add)
                        nc.sync.dma_start(
                            out_d[:].rearrange("p (c f) -> p c f",
                                               c=2)[:, :, 512 * ib:512 * (ib + 1)],
                            out_sb[:].rearrange("p (c f) -> p c f",
                                                c=2)[:, :, 512 * ib:512 * (ib + 1)])
    nc.compile()
    return nc


def _prep_shared(inputs):
    f = np.float32
    bf = ml_dtypes.bfloat16

    def bd(w_dw):
        wr = w_dw.reshape(512, 2, 9)
        Wt = np.zeros((4, 128, 9, 128), f)
        m = np.arange(64)
        for k in range(4):
            blk = wr[128 * k:128 * (k + 1)]        # [128, 2, 9]
            for i in range(2):
                for j in range(2):
                    Wt[k, 2 * m + i, :, 2 * m + j] = blk[2 * m + j, i, :]
        return np.ascontiguousarray(
            Wt.reshape(4, 128, 9 * 128) * 64.0).astype(
                ml_dtypes.float8_e4m3fn)

    w1bd = bd(inputs["w1_dw"])
    w2bd = bd(inputs["w2_dw"])

    pw1 = inputs["w1_pw"][:, :, 0, 0]              # [256, 512]
    pw2 = inputs["w2_pw"][:, :, 0, 0]
    pw1T = np.ascontiguousarray(pw1.T.reshape(4, 128, 256)).astype(bf)
    pw2T = np.ascontiguousarray(pw2.T.reshape(4, 128, 256)).astype(bf)

    gamma = float(inputs["gamma"][0])
    f8t = ml_dtypes.float8_e4m3fn
    wvTg = (inputs["wv"][:, :, 0, 0].T * gamma * 64.0).reshape(2, 128, 256)
    wkT = (inputs["wk"][:, :, 0, 0].T * 64.0).reshape(2, 128, 32)
    wqT = (inputs["wq"][:, :, 0, 0].T * 64.0).reshape(2, 128, 32)
    projw = np.zeros((128, 640), np.float32)
    for ch in range(2):
        projw[:, 320 * ch:320 * ch + 256] = wvTg[ch]
        projw[:, 320 * ch + 256:320 * ch + 288] = wkT[ch]
        projw[:, 320 * ch + 288:320 * ch + 320] = wqT[ch]
    projw = projw.astype(f8t)

    pwall = np.zeros((128, 2048), np.float32)
    for cb, pwT in ((0, pw1T), (1, pw2T)):
        pwall[:, 1024 * cb:1024 * (cb + 1)] = \
            pwT.astype(np.float32).transpose(1, 0, 2).reshape(128, 1024) * 64.0
    pwall = pwall.astype(ml_dtypes.float8_e4m3fn)

    def bn_fold(g, b_, mean, var, pw, b_dw, b_pw):
        s = g / np.sqrt(var + EPS) / 1024.0
        bc = pw @ b_dw + b_pw
        t = s * (bc - mean) + b_
        o = np.zeros((128, 4), f)
        o[:, 0], o[:, 1] = s[0:128], t[0:128]
        o[:, 2], o[:, 3] = s[128:256], t[128:256]
        return o

    consts = np.zeros((128, 11), f)
    consts[:, 0:4] = bn_fold(inputs["bn1_g"], inputs["bn1_b"], inputs["bn1_m"],
                             inputs["bn1_v"], pw1, inputs["b1_dw"],
                             inputs["b1_pw"])
    consts[:, 4:8] = bn_fold(inputs["bn2_g"], inputs["bn2_b"], inputs["bn2_m"],
                             inputs["bn2_v"], pw2, inputs["b2_dw"],
                             inputs["b2_pw"])
    consts[0:32, 8] = inputs["bk"] * 64.0
    consts[32:64, 8] = inputs["bq"] * 64.0
    consts[:, 9] = gamma * inputs["bv"][0:128]
    consts[:, 10] = gamma * inputs["bv"][128:256]

    return dict(w1bd=w1bd, w2bd=w2bd,
                projw=projw, pwall=pwall, consts=consts)


def _prep_core(inputs, shared, b, h):
    bf = ml_dtypes.bfloat16
    x1 = inputs["x1"][b]          # [256, 64, 64]
    x2 = inputs["x2"][b]
    sub = x1 - x2
    cat1 = np.concatenate([sub, x1], axis=0).reshape(4, 128, 64, 64)
    cat2 = np.concatenate([sub, x2], axis=0).reshape(4, 128, 64, 64)

    f8 = ml_dtypes.float8_e4m3fn

    def pack(cc, wbd):
        buf = np.zeros((4, 128, SLOTS, PW8), np.float32)
        if h == 0:
            buf[:, :, 1:34, 1:65] = cc[:, :, 0:33, :]
        else:
            buf[:, :, 0:33, 1:65] = cc[:, :, 31:64, :]
        flat = buf.reshape(4, 128, CAT80).astype(f8)
        cw = np.zeros((4, 128, CATW), f8)
        cw[:, :, 0:CAT80] = flat
        cw[:, :, AREG:AREG + CAT80 - 1] = flat[:, :, 1:]
        cw[:, :, 2 * AREG:] = wbd
        return cw

    x1r = x1.reshape(256, N)[:, QH * h:QH * (h + 1)]   # [256, QH]
    x1h = np.ascontiguousarray(
        np.concatenate([x1r[0:128], x1r[128:256]], axis=1))  # [128, 2*QH]
    return dict(c1w=pack(cat1, shared["w1bd"]),
                c2w=pack(cat2, shared["w2bd"]),
                x1h=x1h)


def kernel(**inputs):
    if "nc" not in _CACHE:
        _CACHE["nc"] = _build_nc()
    nc = _CACHE["nc"]

    inputs = {k: np.ascontiguousarray(np.asarray(v)) for k, v in inputs.items()}
    shared = _prep_shared(inputs)
    in_maps = []
    for core in range(8):
        b, h = core // 2, core % 2
        m = dict(projw=shared["projw"], pwall=shared["pwall"],
                 consts=shared["consts"])
        m.update(_prep_core(inputs, shared, b, h))
        in_maps.append(m)

    res = run_bass_kernel_spmd(nc, in_maps, list(range(8)))
    out = np.empty((4, 256, N), np.float32)
    for core in range(8):
        b, h = core // 2, core % 2
        r = res.results[core]["out"]
        out[b, 0:128, QH * h:QH * (h + 1)] = r[:, 0:QH]
        out[b, 128:256, QH * h:QH * (h + 1)] = r[:, QH:2 * QH]
    return out.reshape(B, C, H, W)


# revision 25
# speedup vs baseline: 1.1720x; 1.1720x over previous
"""CrossAttention kernel for Trainium2, 8 NeuronCores.

Reference pipeline (B=4, C=256, H=W=64, N=4096, d=C//8=32):
  sub = x1 - x2
  x3 = relu(bn1(pw1(dw1([sub, x1]))))      # dw: 3x3 grouped conv (groups=C)
  x4 = relu(bn2(pw2(dw2([sub, x2]))))      # pw: 1x1 512->256
  q = wq@x4 [B,32,N]; k = wk@x3 [B,32,N]; v = wv@x3 [B,256,N]
  attn = softmax(q^T k);  out = gamma * (v @ attn^T) + x1

Sharding: 8 cores = (batch b) x (pixel-half h). Each core computes BOTH
conv paths only for its own pixel half (with a one-row halo), projects
k / v^T / q from its half, then the pair exchanges k and v^T via a
single AllGather so each core runs flash attention for its 2048 queries
over all 4096 keys.

Device-side choices:
  - dw conv on the PE as 9 accumulating block-diagonal [128x128] matmuls
    over a zero-padded 66-col image layout (8-row windows = 512 output
    columns via a strided rhs AP that skips the pad columns).
  - energy is computed transposed, E^T[j, i], with k/q zero-padded to a
    128-deep contraction so the PE streams at full rate.
  - attention accumulation outputs [channel, query] directly (lhsT =
    v^T block, rhs = exp tile): 512-wide matmuls, no PE transposes. The
    softmax denominator comes from an all-ones lhsT matmul which also
    broadcasts it across partitions for the DVE normalize.
  - all inputs packed into few large DMAs (descriptor generation on the
    sync engine costs ~0.7us per dma_start, so count matters).
  - matmul datapath in bf16 (fp32 PSUM accumulation); normalize /
    residual-add in fp32. gamma folded into wv/bv on the host; bn+biases
    folded into per-channel scale/shift applied during PSUM eviction.
"""

import numpy as np
import ml_dtypes

import concourse.bass as bass
import concourse.mybir as mybir
import concourse.tile as tile
from concourse import bacc
from concourse.bass_utils import run_bass_kernel_spmd

F32 = mybir.dt.float32
BF16 = mybir.dt.bfloat16
F8 = mybir.dt.float8e4
U8 = mybir.dt.uint8
PM = mybir.MatmulPerfMode
AF = mybir.ActivationFunctionType
ALU = mybir.AluOpType

B, C, H, W = 4, 256, 64, 64
N = H * W            # 4096 pixels
QH = N // 2          # pixels per core (queries/own keys)
EPS = 1e-5
PW = 66              # padded row width
OFF = 2              # leading pad elements in padded tiles
SLOTS = 34           # 32 data rows + halo/zero rows
CAT_F = OFF + SLOTS * PW + OFF   # 2248 (legacy bf16 layout, unused)
PW8 = 80                         # fp8 padded row stride (16B-aligned)
CAT80 = SLOTS * PW8              # 2720 flat image bytes
AREG = 2736                      # per-copy region (16B-aligned)
CATW = 2 * AREG + 9 * 128        # image copy A | shifted copy B | dw weights
VT = 272             # v^T row stride: 256 channels + pad (16B-aligned for DoubleRow)
PAIRS = [[0, 1], [2, 3], [4, 5], [6, 7]]

_CACHE = {}


def _build_nc():
    nc = bacc.Bacc("TRN2", target_bir_lowering=False, debug=False, num_devices=8)

    c1w_d = nc.dram_tensor("c1w", [4, 128, CATW], F8, kind="ExternalInput")
    c2w_d = nc.dram_tensor("c2w", [4, 128, CATW], F8, kind="ExternalInput")
    x1h_d = nc.dram_tensor("x1h", [128, 2 * QH], F32, kind="ExternalInput")
    consts_d = nc.dram_tensor("consts", [128, 11], F32, kind="ExternalInput")
    projw_d = nc.dram_tensor("projw", [128, 640], BF16, kind="ExternalInput")
    pwall_d = nc.dram_tensor("pwall", [128, 2048], F8, kind="ExternalInput")
    out_d = nc.dram_tensor("out", [128, 2 * QH], F32, kind="ExternalOutput")

    # collective bounce buffers, in bytes: v^T fp8 (128x4128) + k bf16 (32x2048)
    VSZ = 128 * 16 * VT              # v^T bytes
    KVN = VSZ + 32 * QH * 2          # + k bytes = 659456
    kvown_d = nc.dram_tensor("kvown_b", [KVN], U8)
    kvfull_d = nc.dram_tensor("kvfull_b", [2 * KVN], U8)

    with tile.TileContext(nc) as tc:
        with tc.tile_pool(name="persist", bufs=1) as pp:
            catw = [[pp.tile([128, CATW], F8, name=f"cw{b}_{k}",
                             tag=f"cw{b}_{k}") for k in range(4)]
                    for b in range(2)]
            x3o = [pp.tile([128, QH], BF16, name=f"x3o_{m}", tag=f"x3o_{m}")
                   for m in range(2)]
            x4 = [pp.tile([128, QH], BF16, name=f"x4_{m}", tag=f"x4_{m}")
                  for m in range(2)]
            consts = pp.tile([128, 11], F32, name="consts", tag="consts")
            projw = pp.tile([128, 640], BF16, name="projw", tag="projw")
            pwall = pp.tile([128, 2048], F8, name="pwall", tag="pwall")
            k_own = pp.tile([128, QH], BF16, name="k_own", tag="k_own")
            vto = pp.tile([128, 16 * VT], F8, name="vto", tag="vto")
            k_sb = pp.tile([128, N], BF16, name="k_sb", tag="k_sb")
            q_sb = pp.tile([128, QH], BF16, name="q_sb", tag="q_sb")
            vta = pp.tile([128, 32 * VT], F8, name="vta", tag="vta")
            x1h = pp.tile([128, 2 * QH], F32, name="x1h", tag="x1h")
            out_sb = pp.tile([128, 2 * QH], F32, name="osb", tag="osb")
            ones_sb = pp.tile([128, 256], F8, name="ones", tag="ones")
            # zero rows 32:128 so energy matmuls can use full 128-row lhsT
            # (avoids the PE small-tile row-group slowdown)
            nc.gpsimd.memset(k_sb[:], 0.0)
            nc.gpsimd.memset(q_sb[:], 0.0)
            nc.gpsimd.memset(ones_sb[:], 1.0)

            # input DMAs in priority order (conv1, consts, conv2, residual)
            for k in range(4):
                nc.sync.dma_start(catw[0][k][:], c1w_d[k])
            nc.sync.dma_start(consts[:], consts_d[:])
            nc.sync.dma_start(pwall[:], pwall_d[:])
            nc.sync.dma_start(projw[:], projw_d[:])
            for k in range(4):
                nc.sync.dma_start(catw[1][k][:], c2w_d[k])
            nc.sync.dma_start(x1h[:], x1h_d[:])

            pwr = pwall[:].rearrange("p (b k m) -> p b k m", b=2, k=4)

            def conv_block(cb, cw, bno, xout, after_window=None):
                # dw in bf16 (shifted-window APs rule out DoubleRow); y1
                # evicted x16 into fp8 so the 1x1 pw conv runs as fp8
                # DoubleRow k-tile pairs (weights x64; bn scale /1024)
                WO = 2 * AREG
                with tc.tile_pool(name="conv_y", bufs=2) as cyb, \
                     tc.tile_pool(name="conv_ps", bufs=2, space="PSUM") as cps:
                    for w in range(4):
                        y1w = cyb.tile([128, 2048], F8, name="y1w", tag="y1w")
                        y1r = y1w[:].rearrange("p (k f) -> p k f", k=4)
                        for k in range(4):
                            ab = cw[k][:, 0:WO].rearrange("p (c f) -> p c f",
                                                          c=2)
                            for sub in range(2):
                                w4 = 2 * w + sub
                                ps = cps.tile([128, 320], F32, name="dwps",
                                              tag="dwps")
                                for dr in range(3):
                                    s = (4 * w4 + dr) * PW8
                                    nc.tensor.matmul(
                                        ps[:],
                                        cw[k][:, WO + 384 * dr:
                                              WO + 384 * dr + 256]
                                        .rearrange("p (c f) -> p c f", c=2),
                                        ab[:, :, s:s + 320],
                                        start=(dr == 0), stop=False,
                                        perf_mode=PM.DoubleRow)
                                    nc.tensor.matmul(
                                        ps[:],
                                        cw[k][:, WO + 384 * dr + 256:
                                              WO + 384 * dr + 384],
                                        cw[k][:, s + 2:s + 322],
                                        start=False, stop=(dr == 2))
                                nc.scalar.activation(
                                    y1w[:, 512 * k + 256 * sub:
                                        512 * k + 256 * (sub + 1)],
                                    ps[:, 0:320].rearrange(
                                        "p (r c) -> p r c", r=4,
                                        c=PW8)[:, :, 0:64],
                                    AF.Copy, scale=0.25)
                        for m in range(2):
                            pp2 = cps.tile([128, 512], F32, name="pwps", tag="pwps")
                            for kp in range(2):
                                nc.tensor.matmul(
                                    pp2[:],
                                    pwr[:, cb, 2 * kp:2 * kp + 2,
                                        128 * m:128 * (m + 1)],
                                    y1r[:, 2 * kp:2 * kp + 2, :],
                                    start=(kp == 0), stop=(kp == 1),
                                    perf_mode=PM.DoubleRow)
                            nc.scalar.activation(
                                xout[m][:, 512 * w:512 * (w + 1)], pp2[:],
                                AF.Relu,
                                bias=consts[:, bno + 2 * m + 1:bno + 2 * m + 2],
                                scale=consts[:, bno + 2 * m:bno + 2 * m + 1])
                        if after_window is not None:
                            after_window(w)

            # projections interleaved into conv1's window loop: k / v^T
            # blocks only need that window's x3 columns, so the pair
            # exchange can trigger right at conv1's end
            pps_c = tc.tile_pool(name="proj_ps", bufs=2, space="PSUM")
            pps = pps_c.__enter__()

            def proj1_window(s):
                ps = pps.tile([128, 512], F32, name="kqps", tag="kqps")
                for ch in range(2):
                    nc.tensor.matmul(ps[0:32, :],
                                     projw[:, 320 * ch + 256:320 * ch + 288],
                                     x3o[ch][:, 512 * s:512 * (s + 1)],
                                     start=(ch == 0), stop=(ch == 1))
                nc.scalar.activation(k_own[0:32, 512 * s:512 * (s + 1)],
                                     ps[0:32, :], AF.Identity,
                                     bias=consts[0:32, 8:9])
                for j in range(4 * s, 4 * s + 4):
                    ps = pps.tile([128, 256], F32, name="vtps", tag="vtps")
                    for ch in range(2):
                        nc.tensor.matmul(ps[:], x3o[ch][:, 128 * j:128 * (j + 1)],
                                         projw[:, 320 * ch:320 * ch + 256],
                                         start=(ch == 0), stop=(ch == 1))
                    nc.scalar.activation(vto[:, VT * j:VT * j + 256], ps[:],
                                         AF.Copy)

            conv_block(0, catw[0], 0, x3o, after_window=proj1_window)

            if True:
                # ship own k / v^T, single AllGather for the pair
                nc.sync.dma_start(
                    kvown_d[0:VSZ].rearrange("(p f) -> p f", p=128),
                    vto[:].bitcast(U8))
                nc.sync.dma_start(
                    kvown_d[VSZ:KVN].rearrange("(p f) -> p f", p=32),
                    k_own[0:32, :].bitcast(U8))
                nc.gpsimd.collective_compute(
                    "AllGather", ALU.bypass, replica_groups=PAIRS,
                    ins=[kvown_d[:].opt()], outs=[kvfull_d[:].opt()])
                for m in range(2):
                    o = m * KVN
                    nc.sync.dma_start(
                        vta[:, 16 * VT * m:16 * VT * (m + 1)].bitcast(U8),
                        kvfull_d[o:o + VSZ].rearrange("(p f) -> p f", p=128))
                    nc.sync.dma_start(
                        k_sb[0:32, QH * m:QH * (m + 1)].bitcast(U8),
                        kvfull_d[o + VSZ:o + KVN].rearrange("(p f) -> p f",
                                                            p=32))

                # conv2 + q overlap with the collective
                def proj2_window(s):
                    ps = pps.tile([128, 512], F32, name="kqps", tag="kqps")
                    for ch in range(2):
                        nc.tensor.matmul(ps[0:32, :],
                                         projw[:, 320 * ch + 288:320 * ch + 320],
                                         x4[ch][:, 512 * s:512 * (s + 1)],
                                         start=(ch == 0), stop=(ch == 1))
                    nc.scalar.activation(q_sb[0:32, 512 * s:512 * (s + 1)],
                                         ps[0:32, :], AF.Identity,
                                         bias=consts[32:64, 8:9])

                conv_block(1, catw[1], 4, x4, after_window=proj2_window)
            pps_c.__exit__(None, None, None)

            # ---- flash attention (output computed pre-transposed) ----
            # single flattened pipeline over 64 key-block pairs (4 query
            # blocks x 16) so the PE/scalar pipeline never drains at query
            # block boundaries; normalize uses a fused divide stt so acc
            # banks free quickly
            with tc.tile_pool(name="att_sb", bufs=2) as asb, \
                 tc.tile_pool(name="acc_ps", bufs=1, space="PSUM") as accp, \
                 tc.tile_pool(name="dps_ps", bufs=2, space="PSUM") as dpsp, \
                 tc.tile_pool(name="e_ps", bufs=2, space="PSUM") as epsp:
                vv = vta[:].rearrange("p (j v) -> p j v", j=32)
                ones2 = ones_sb[:].rearrange("p (k f) -> p k f", k=2)
                eps_t = {}
                acc = None
                dps = None

                def energy_pair(g):
                    ib = g // 16
                    p = g % 16
                    eps_t[g] = epsp.tile([128, 1024], F32, name="eps",
                                         tag="eps")
                    for h in range(2):
                        nc.tensor.matmul(
                            eps_t[g][:, 512 * h:512 * (h + 1)],
                            k_sb[:, 128 * (2 * p + h):128 * (2 * p + h + 1)],
                            q_sb[:, 512 * ib:512 * (ib + 1)],
                            start=True, stop=True)

                for g in range(64):
                    ib, p = g // 16, g % 16
                    if g == 0:
                        energy_pair(0)
                    if p == 0:
                        acc = [accp.tile([128, 512], F32, name=f"acc{c}",
                                         tag=f"acc{c}") for c in range(2)]
                        dps = dpsp.tile([128, 512], F32, name="dps", tag="dps")
                    expair = asb.tile([128, 1024], F8, name="ex", tag="ex")
                    nc.scalar.activation(expair[:], eps_t[g][:], AF.Exp)
                    if g + 1 < 64:
                        energy_pair(g + 1)
                    eps_t.pop(g - 1, None)
                    rhs2 = expair[:].rearrange("p (k f) -> p k f", k=2)
                    nc.tensor.matmul(acc[0][:], vv[:, 2 * p:2 * p + 2, 0:128],
                                     rhs2, start=(p == 0), stop=(p == 15),
                                     perf_mode=PM.DoubleRow)
                    nc.tensor.matmul(acc[1][:],
                                     vv[:, 2 * p:2 * p + 2, 128:256],
                                     rhs2, start=(p == 0), stop=(p == 15),
                                     perf_mode=PM.DoubleRow)
                    nc.tensor.matmul(dps[:], ones2, rhs2,
                                     start=(p == 0), stop=(p == 15),
                                     perf_mode=PM.DoubleRow)
                    if p == 15:
                        # normalize: PSUM-reading ops first so acc banks
                        # free before the next query block needs them
                        rec = asb.tile([128, 512], F32, name="rec", tag="rec")
                        nc.vector.reciprocal_approx_fast(rec[:], dps[:])
                        tmp = [asb.tile([128, 512], BF16, name=f"tmp{c}",
                                        tag=f"tmp{c}") for c in range(2)]
                        for ch in range(2):
                            nc.vector.scalar_tensor_tensor(
                                tmp[ch][:], acc[ch][:], 1.0, rec[:],
                                ALU.mult, ALU.mult)
                        for ch in range(2):
                            nc.vector.scalar_tensor_tensor(
                                out_sb[:,
                                       QH * ch + 512 * ib:QH * ch + 512 * (ib + 1)],
                                tmp[ch][:], consts[:, 9 + ch:10 + ch],
                                x1h[:,
                                    QH * ch + 512 * ib:QH * ch + 512 * (ib + 1)],
                                ALU.add, ALU.add)
                        nc.sync.dma_start(
                            out_d[:].rearrange("p (c f) -> p c f",
                                               c=2)[:, :, 512 * ib:512 * (ib + 1)],
                            out_sb[:].rearrange("p (c f) -> p c f",
                                                c=2)[:, :, 512 * ib:512 * (ib + 1)])
    nc.compile()
    return nc


def _prep_shared(inputs):
    f = np.float32
    bf = ml_dtypes.bfloat16

    def bd(w_dw):
        wr = w_dw.reshape(512, 2, 9)
        Wt = np.zeros((4, 128, 9, 128), f)
        m = np.arange(64)
        for k in range(4):
            blk = wr[128 * k:128 * (k + 1)]        # [128, 2, 9]
            for i in range(2):
                for j in range(2):
                    Wt[k, 2 * m + i, :, 2 * m + j] = blk[2 * m + j, i, :]
        return np.ascontiguousarray(
            Wt.reshape(4, 128, 9 * 128) * 64.0).astype(
                ml_dtypes.float8_e4m3fn)

    w1bd = bd(inputs["w1_dw"])
    w2bd = bd(inputs["w2_dw"])

    pw1 = inputs["w1_pw"][:, :, 0, 0]              # [256, 512]
    pw2 = inputs["w2_pw"][:, :, 0, 0]
    pw1T = np.ascontiguousarray(pw1.T.reshape(4, 128, 256)).astype(bf)
    pw2T = np.ascontiguousarray(pw2.T.reshape(4, 128, 256)).astype(bf)

    gamma = float(inputs["gamma"][0])
    wvTg = (inputs["wv"][:, :, 0, 0].T * gamma).reshape(2, 128, 256).astype(bf)
    wkT = inputs["wk"][:, :, 0, 0].T.reshape(2, 128, 32).astype(bf)
    wqT = inputs["wq"][:, :, 0, 0].T.reshape(2, 128, 32).astype(bf)
    projw = np.zeros((128, 640), bf)
    for ch in range(2):
        projw[:, 320 * ch:320 * ch + 256] = wvTg[ch]
        projw[:, 320 * ch + 256:320 * ch + 288] = wkT[ch]
        projw[:, 320 * ch + 288:320 * ch + 320] = wqT[ch]

    pwall = np.zeros((128, 2048), np.float32)
    for cb, pwT in ((0, pw1T), (1, pw2T)):
        pwall[:, 1024 * cb:1024 * (cb + 1)] = \
            pwT.astype(np.float32).transpose(1, 0, 2).reshape(128, 1024) * 64.0
    pwall = pwall.astype(ml_dtypes.float8_e4m3fn)

    def bn_fold(g, b_, mean, var, pw, b_dw, b_pw):
        s = g / np.sqrt(var + EPS) / 1024.0
        bc = pw @ b_dw + b_pw
        t = s * (bc - mean) + b_
        o = np.zeros((128, 4), f)
        o[:, 0], o[:, 1] = s[0:128], t[0:128]
        o[:, 2], o[:, 3] = s[128:256], t[128:256]
        return o

    consts = np.zeros((128, 11), f)
    consts[:, 0:4] = bn_fold(inputs["bn1_g"], inputs["bn1_b"], inputs["bn1_m"],
                             inputs["bn1_v"], pw1, inputs["b1_dw"],
                             inputs["b1_pw"])
    consts[:, 4:8] = bn_fold(inputs["bn2_g"], inputs["bn2_b"], inputs["bn2_m"],
                             inputs["bn2_v"], pw2, inputs["b2_dw"],
                             inputs["b2_pw"])
    consts[0:32, 8] = inputs["bk"]
    consts[32:64, 8] = inputs["bq"]
    consts[:, 9] = gamma * inputs["bv"][0:128]
    consts[:, 10] = gamma * inputs["bv"][128:256]

    return dict(w1bd=w1bd, w2bd=w2bd,
                projw=projw, pwall=pwall, consts=consts)


def _prep_core(inputs, shared, b, h):
    bf = ml_dtypes.bfloat16
    x1 = inputs["x1"][b]          # [256, 64, 64]
    x2 = inputs["x2"][b]
    sub = x1 - x2
    cat1 = np.concatenate([sub, x1], axis=0).reshape(4, 128, 64, 64)
    cat2 = np.concatenate([sub, x2], axis=0).reshape(4, 128, 64, 64)

    f8 = ml_dtypes.float8_e4m3fn

    def pack(cc, wbd):
        buf = np.zeros((4, 128, SLOTS, PW8), np.float32)
        if h == 0:
            buf[:, :, 1:34, 1:65] = cc[:, :, 0:33, :]
        else:
            buf[:, :, 0:33, 1:65] = cc[:, :, 31:64, :]
        flat = buf.reshape(4, 128, CAT80).astype(f8)
        cw = np.zeros((4, 128, CATW), f8)
        cw[:, :, 0:CAT80] = flat
        cw[:, :, AREG:AREG + CAT80 - 1] = flat[:, :, 1:]
        cw[:, :, 2 * AREG:] = wbd
        return cw

    x1r = x1.reshape(256, N)[:, QH * h:QH * (h + 1)]   # [256, QH]
    x1h = np.ascontiguousarray(
        np.concatenate([x1r[0:128], x1r[128:256]], axis=1))  # [128, 2*QH]
    return dict(c1w=pack(cat1, shared["w1bd"]),
                c2w=pack(cat2, shared["w2bd"]),
                x1h=x1h)


def kernel(**inputs):
    if "nc" not in _CACHE:
        _CACHE["nc"] = _build_nc()
    nc = _CACHE["nc"]

    inputs = {k: np.ascontiguousarray(np.asarray(v)) for k, v in inputs.items()}
    shared = _prep_shared(inputs)
    in_maps = []
    for core in range(8):
        b, h = core // 2, core % 2
        m = dict(projw=shared["projw"], pwall=shared["pwall"],
                 consts=shared["consts"])
        m.update(_prep_core(inputs, shared, b, h))
        in_maps.append(m)

    res = run_bass_kernel_spmd(nc, in_maps, list(range(8)))
    out = np.empty((4, 256, N), np.float32)
    for core in range(8):
        b, h = core // 2, core % 2
        r = res.results[core]["out"]
        out[b, 0:128, QH * h:QH * (h + 1)] = r[:, 0:QH]
        out[b, 128:256, QH * h:QH * (h + 1)] = r[:, QH:2 * QH]
    return out.reshape(B, C, H, W)


# revision 27
# speedup vs baseline: 1.2417x; 1.0594x over previous
"""CrossAttention kernel for Trainium2, 8 NeuronCores.

Reference pipeline (B=4, C=256, H=W=64, N=4096, d=C//8=32):
  sub = x1 - x2
  x3 = relu(bn1(pw1(dw1([sub, x1]))))      # dw: 3x3 grouped conv (groups=C)
  x4 = relu(bn2(pw2(dw2([sub, x2]))))      # pw: 1x1 512->256
  q = wq@x4 [B,32,N]; k = wk@x3 [B,32,N]; v = wv@x3 [B,256,N]
  attn = softmax(q^T k);  out = gamma * (v @ attn^T) + x1

Sharding: 8 cores = (batch b) x (pixel-half h). Each core computes BOTH
conv paths only for its own pixel half (with a one-row halo), projects
k / v^T / q from its half, then the pair exchanges k and v^T via a
single AllGather so each core runs flash attention for its 2048 queries
over all 4096 keys.

Device-side choices:
  - dw conv on the PE as 9 accumulating block-diagonal [128x128] matmuls
    over a zero-padded 66-col image layout (8-row windows = 512 output
    columns via a strided rhs AP that skips the pad columns).
  - energy is computed transposed, E^T[j, i], with k/q zero-padded to a
    128-deep contraction so the PE streams at full rate.
  - attention accumulation outputs [channel, query] directly (lhsT =
    v^T block, rhs = exp tile): 512-wide matmuls, no PE transposes. The
    softmax denominator comes from an all-ones lhsT matmul which also
    broadcasts it across partitions for the DVE normalize.
  - all inputs packed into few large DMAs (descriptor generation on the
    sync engine costs ~0.7us per dma_start, so count matters).
  - matmul datapath in bf16 (fp32 PSUM accumulation); normalize /
    residual-add in fp32. gamma folded into wv/bv on the host; bn+biases
    folded into per-channel scale/shift applied during PSUM eviction.
"""

import numpy as np
import ml_dtypes

import concourse.bass as bass
import concourse.mybir as mybir
import concourse.tile as tile
from concourse import bacc
from concourse.bass_utils import run_bass_kernel_spmd

F32 = mybir.dt.float32
BF16 = mybir.dt.bfloat16
F8 = mybir.dt.float8e4
U8 = mybir.dt.uint8
PM = mybir.MatmulPerfMode
AF = mybir.ActivationFunctionType
ALU = mybir.AluOpType

B, C, H, W = 4, 256, 64, 64
N = H * W            # 4096 pixels
QH = N // 2          # pixels per core (queries/own keys)
EPS = 1e-5
PW = 66              # padded row width
OFF = 2              # leading pad elements in padded tiles
SLOTS = 34           # 32 data rows + halo/zero rows
CAT_F = OFF + SLOTS * PW + OFF   # 2248 (legacy bf16 layout, unused)
PW8 = 80                         # fp8 padded row stride (16B-aligned)
CAT80 = SLOTS * PW8              # 2720 flat image bytes
AREG = 2736                      # per-copy region (16B-aligned)
CATW = 2 * AREG + 9 * 128        # image copy A | shifted copy B | dw weights
VT = 272             # v^T row stride: 256 channels + pad (16B-aligned for DoubleRow)
PAIRS = [[0, 1], [2, 3], [4, 5], [6, 7]]

_CACHE = {}


def _build_nc():
    nc = bacc.Bacc("TRN2", target_bir_lowering=False, debug=False, num_devices=8)

    c1w_d = nc.dram_tensor("c1w", [4, 128, CATW], F8, kind="ExternalInput")
    c2w_d = nc.dram_tensor("c2w", [4, 128, CATW], F8, kind="ExternalInput")
    x1h_d = nc.dram_tensor("x1h", [128, 2 * QH], F32, kind="ExternalInput")
    consts_d = nc.dram_tensor("consts", [128, 11], F32, kind="ExternalInput")
    projw_d = nc.dram_tensor("projw", [128, 640], BF16, kind="ExternalInput")
    pwall_d = nc.dram_tensor("pwall", [128, 2048], F8, kind="ExternalInput")
    out_d = nc.dram_tensor("out", [128, 2 * QH], F32, kind="ExternalOutput")

    # collective bounce buffers, in bytes: v^T fp8 (128x4128) + k bf16 (32x2048)
    VSZ = 128 * 16 * VT              # v^T bytes
    KVN = VSZ + 32 * QH * 2          # + k bytes = 659456
    kvown_d = nc.dram_tensor("kvown_b", [KVN], U8)
    kvfull_d = nc.dram_tensor("kvfull_b", [2 * KVN], U8)

    with tile.TileContext(nc) as tc:
        with tc.tile_pool(name="persist", bufs=1) as pp:
            catw = [[pp.tile([128, CATW], F8, name=f"cw{b}_{k}",
                             tag=f"cw{b}_{k}") for k in range(4)]
                    for b in range(2)]
            x3o = [pp.tile([128, QH], BF16, name=f"x3o_{m}", tag=f"x3o_{m}")
                   for m in range(2)]
            x4 = [pp.tile([128, QH], BF16, name=f"x4_{m}", tag=f"x4_{m}")
                  for m in range(2)]
            consts = pp.tile([128, 11], F32, name="consts", tag="consts")
            projw = pp.tile([128, 640], BF16, name="projw", tag="projw")
            pwall = pp.tile([128, 2048], F8, name="pwall", tag="pwall")
            k_own = pp.tile([128, QH], BF16, name="k_own", tag="k_own")
            vto = pp.tile([128, 16 * VT], F8, name="vto", tag="vto")
            k_sb = pp.tile([128, N], BF16, name="k_sb", tag="k_sb")
            q_sb = pp.tile([128, QH], BF16, name="q_sb", tag="q_sb")
            vta = pp.tile([128, 32 * VT], F8, name="vta", tag="vta")
            x1h = pp.tile([128, 2 * QH], F32, name="x1h", tag="x1h")
            out_sb = pp.tile([128, 2 * QH], F32, name="osb", tag="osb")
            ones_sb = pp.tile([128, 256], F8, name="ones", tag="ones")
            # zero rows 32:128 so energy matmuls can use full 128-row lhsT
            # (avoids the PE small-tile row-group slowdown)
            nc.gpsimd.memset(k_sb[:], 0.0)
            nc.gpsimd.memset(q_sb[:], 0.0)
            nc.gpsimd.memset(ones_sb[:], 1.0)

            # input DMAs in priority order (conv1, consts, conv2, residual)
            for k in range(4):
                nc.sync.dma_start(catw[0][k][:], c1w_d[k])
            nc.sync.dma_start(consts[:], consts_d[:])
            nc.sync.dma_start(pwall[:], pwall_d[:])
            nc.sync.dma_start(projw[:], projw_d[:])
            for k in range(4):
                nc.sync.dma_start(catw[1][k][:], c2w_d[k])
            nc.sync.dma_start(x1h[:], x1h_d[:])

            pwr = pwall[:].rearrange("p (b k m) -> p b k m", b=2, k=4)

            def conv_block(cb, cw, bno, xout, after_window=None):
                # dw in bf16 (shifted-window APs rule out DoubleRow); y1
                # evicted x16 into fp8 so the 1x1 pw conv runs as fp8
                # DoubleRow k-tile pairs (weights x64; bn scale /1024)
                WO = 2 * AREG
                with tc.tile_pool(name="conv_y", bufs=2) as cyb, \
                     tc.tile_pool(name="conv_ps", bufs=2, space="PSUM") as cps:
                    for w in range(4):
                        y1w = cyb.tile([128, 2048], F8, name="y1w", tag="y1w")
                        y1r = y1w[:].rearrange("p (k f) -> p k f", k=4)
                        for k in range(4):
                            ab = cw[k][:, 0:WO].rearrange("p (c f) -> p c f",
                                                          c=2)
                            for sub in range(2):
                                w4 = 2 * w + sub
                                ps = cps.tile([128, 320], F32, name="dwps",
                                              tag="dwps")
                                for dr in range(3):
                                    s = (4 * w4 + dr) * PW8
                                    nc.tensor.matmul(
                                        ps[:],
                                        cw[k][:, WO + 384 * dr:
                                              WO + 384 * dr + 256]
                                        .rearrange("p (c f) -> p c f", c=2),
                                        ab[:, :, s:s + 320],
                                        start=(dr == 0), stop=False,
                                        perf_mode=PM.DoubleRow)
                                    nc.tensor.matmul(
                                        ps[:],
                                        cw[k][:, WO + 384 * dr + 256:
                                              WO + 384 * dr + 384],
                                        cw[k][:, s + 2:s + 322],
                                        start=False, stop=(dr == 2))
                                nc.scalar.activation(
                                    y1w[:, 512 * k + 256 * sub:
                                        512 * k + 256 * (sub + 1)],
                                    ps[:, 0:320].rearrange(
                                        "p (r c) -> p r c", r=4,
                                        c=PW8)[:, :, 0:64],
                                    AF.Copy, scale=0.25)
                        for m in range(2):
                            pp2 = cps.tile([128, 512], F32, name="pwps", tag="pwps")
                            for kp in range(2):
                                nc.tensor.matmul(
                                    pp2[:],
                                    pwr[:, cb, 2 * kp:2 * kp + 2,
                                        128 * m:128 * (m + 1)],
                                    y1r[:, 2 * kp:2 * kp + 2, :],
                                    start=(kp == 0), stop=(kp == 1),
                                    perf_mode=PM.DoubleRow)
                            nc.scalar.activation(
                                xout[m][:, 512 * w:512 * (w + 1)], pp2[:],
                                AF.Relu,
                                bias=consts[:, bno + 2 * m + 1:bno + 2 * m + 2],
                                scale=consts[:, bno + 2 * m:bno + 2 * m + 1])
                        if after_window is not None:
                            after_window(w)

            # projections interleaved into conv1's window loop: k / v^T
            # blocks only need that window's x3 columns, so the pair
            # exchange can trigger right at conv1's end
            pps_c = tc.tile_pool(name="proj_ps", bufs=2, space="PSUM")
            pps = pps_c.__enter__()

            def proj1_window(s):
                ps = pps.tile([128, 512], F32, name="kqps", tag="kqps")
                for ch in range(2):
                    nc.tensor.matmul(ps[0:32, :],
                                     projw[:, 320 * ch + 256:320 * ch + 288],
                                     x3o[ch][:, 512 * s:512 * (s + 1)],
                                     start=(ch == 0), stop=(ch == 1))
                nc.scalar.activation(k_own[0:32, 512 * s:512 * (s + 1)],
                                     ps[0:32, :], AF.Identity,
                                     bias=consts[0:32, 8:9])
                for jp in range(2 * s, 2 * s + 2):
                    ps = pps.tile([128, 512], F32, name="vtps", tag="vtps")
                    for u in range(2):
                        for ch in range(2):
                            nc.tensor.matmul(
                                ps[:, 256 * u:256 * (u + 1)],
                                x3o[ch][:, 128 * (2 * jp + u):
                                        128 * (2 * jp + u + 1)],
                                projw[:, 320 * ch:320 * ch + 256],
                                start=(ch == 0), stop=(ch == 1))
                    nc.scalar.activation(
                        vto[:].rearrange("p (j v) -> p j v",
                                         j=16)[:, 2 * jp:2 * jp + 2, 0:256],
                        ps[:].rearrange("p (u v) -> p u v", u=2),
                        AF.Copy)

            conv_block(0, catw[0], 0, x3o, after_window=proj1_window)

            if True:
                # ship own k / v^T, single AllGather for the pair
                nc.sync.dma_start(
                    kvown_d[0:VSZ].rearrange("(p f) -> p f", p=128),
                    vto[:].bitcast(U8))
                nc.sync.dma_start(
                    kvown_d[VSZ:KVN].rearrange("(p f) -> p f", p=32),
                    k_own[0:32, :].bitcast(U8))
                nc.gpsimd.collective_compute(
                    "AllGather", ALU.bypass, replica_groups=PAIRS,
                    ins=[kvown_d[:].opt()], outs=[kvfull_d[:].opt()])
                for m in range(2):
                    o = m * KVN
                    nc.sync.dma_start(
                        vta[:, 16 * VT * m:16 * VT * (m + 1)].bitcast(U8),
                        kvfull_d[o:o + VSZ].rearrange("(p f) -> p f", p=128))
                    nc.sync.dma_start(
                        k_sb[0:32, QH * m:QH * (m + 1)].bitcast(U8),
                        kvfull_d[o + VSZ:o + KVN].rearrange("(p f) -> p f",
                                                            p=32))

                # conv2 + q overlap with the collective
                def proj2_window(s):
                    ps = pps.tile([128, 512], F32, name="kqps", tag="kqps")
                    for ch in range(2):
                        nc.tensor.matmul(ps[0:32, :],
                                         projw[:, 320 * ch + 288:320 * ch + 320],
                                         x4[ch][:, 512 * s:512 * (s + 1)],
                                         start=(ch == 0), stop=(ch == 1))
                    nc.scalar.activation(q_sb[0:32, 512 * s:512 * (s + 1)],
                                         ps[0:32, :], AF.Identity,
                                         bias=consts[32:64, 8:9])

                conv_block(1, catw[1], 4, x4, after_window=proj2_window)
            pps_c.__exit__(None, None, None)

            # ---- flash attention (output computed pre-transposed) ----
            # single flattened pipeline over 64 key-block pairs (4 query
            # blocks x 16) so the PE/scalar pipeline never drains at query
            # block boundaries; normalize uses a fused divide stt so acc
            # banks free quickly
            with tc.tile_pool(name="att_sb", bufs=3) as asb, \
                 tc.tile_pool(name="acc_ps", bufs=1, space="PSUM") as accp, \
                 tc.tile_pool(name="dps_ps", bufs=2, space="PSUM") as dpsp, \
                 tc.tile_pool(name="e_ps", bufs=2, space="PSUM") as epsp:
                vv = vta[:].rearrange("p (j v) -> p j v", j=32)
                ones2 = ones_sb[:].rearrange("p (k f) -> p k f", k=2)
                eps_t = {}
                acc = None
                dps = None

                def energy_pair(g):
                    ib = g // 16
                    p = g % 16
                    eps_t[g] = epsp.tile([128, 1024], F32, name="eps",
                                         tag="eps")
                    for h in range(2):
                        nc.tensor.matmul(
                            eps_t[g][:, 512 * h:512 * (h + 1)],
                            k_sb[:, 128 * (2 * p + h):128 * (2 * p + h + 1)],
                            q_sb[:, 512 * ib:512 * (ib + 1)],
                            start=True, stop=True)

                for g in range(64):
                    ib, p = g // 16, g % 16
                    if g == 0:
                        energy_pair(0)
                    if p == 0:
                        acc = [accp.tile([128, 512], F32, name=f"acc{c}",
                                         tag=f"acc{c}") for c in range(2)]
                        dps = dpsp.tile([128, 512], F32, name="dps", tag="dps")
                    expair = asb.tile([128, 1024], F8, name="ex", tag="ex")
                    nc.scalar.activation(expair[:], eps_t[g][:], AF.Exp)
                    if g + 1 < 64:
                        energy_pair(g + 1)
                    eps_t.pop(g - 1, None)
                    rhs2 = expair[:].rearrange("p (k f) -> p k f", k=2)
                    nc.tensor.matmul(acc[0][:], vv[:, 2 * p:2 * p + 2, 0:128],
                                     rhs2, start=(p == 0), stop=(p == 15),
                                     perf_mode=PM.DoubleRow)
                    nc.tensor.matmul(acc[1][:],
                                     vv[:, 2 * p:2 * p + 2, 128:256],
                                     rhs2, start=(p == 0), stop=(p == 15),
                                     perf_mode=PM.DoubleRow)
                    nc.tensor.matmul(dps[:], ones2, rhs2,
                                     start=(p == 0), stop=(p == 15),
                                     perf_mode=PM.DoubleRow)
                    if p == 15:
                        # normalize: PSUM-reading ops first so acc banks
                        # free before the next query block needs them
                        rec = asb.tile([128, 512], F32, name="rec", tag="rec")
                        nc.vector.reciprocal_approx_fast(rec[:], dps[:])
                        tmp = [asb.tile([128, 512], BF16, name=f"tmp{c}",
                                        tag=f"tmp{c}") for c in range(2)]
                        for ch in range(2):
                            nc.vector.scalar_tensor_tensor(
                                tmp[ch][:], acc[ch][:], 1.0, rec[:],
                                ALU.mult, ALU.mult)
                        for ch in range(2):
                            nc.vector.scalar_tensor_tensor(
                                out_sb[:,
                                       QH * ch + 512 * ib:QH * ch + 512 * (ib + 1)],
                                tmp[ch][:], consts[:, 9 + ch:10 + ch],
                                x1h[:,
                                    QH * ch + 512 * ib:QH * ch + 512 * (ib + 1)],
                                ALU.add, ALU.add)
                        nc.sync.dma_start(
                            out_d[:].rearrange("p (c f) -> p c f",
                                               c=2)[:, :, 512 * ib:512 * (ib + 1)],
                            out_sb[:].rearrange("p (c f) -> p c f",
                                                c=2)[:, :, 512 * ib:512 * (ib + 1)])
    nc.compile()
    return nc


def _prep_shared(inputs):
    f = np.float32
    bf = ml_dtypes.bfloat16

    def bd(w_dw):
        wr = w_dw.reshape(512, 2, 9)
        Wt = np.zeros((4, 128, 9, 128), f)
        m = np.arange(64)
        for k in range(4):
            blk = wr[128 * k:128 * (k + 1)]        # [128, 2, 9]
            for i in range(2):
                for j in range(2):
                    Wt[k, 2 * m + i, :, 2 * m + j] = blk[2 * m + j, i, :]
        return np.ascontiguousarray(
            Wt.reshape(4, 128, 9 * 128) * 64.0).astype(
                ml_dtypes.float8_e4m3fn)

    w1bd = bd(inputs["w1_dw"])
    w2bd = bd(inputs["w2_dw"])

    pw1 = inputs["w1_pw"][:, :, 0, 0]              # [256, 512]
    pw2 = inputs["w2_pw"][:, :, 0, 0]
    pw1T = np.ascontiguousarray(pw1.T.reshape(4, 128, 256)).astype(bf)
    pw2T = np.ascontiguousarray(pw2.T.reshape(4, 128, 256)).astype(bf)

    gamma = float(inputs["gamma"][0])
    wvTg = (inputs["wv"][:, :, 0, 0].T * gamma).reshape(2, 128, 256).astype(bf)
    wkT = inputs["wk"][:, :, 0, 0].T.reshape(2, 128, 32).astype(bf)
    wqT = inputs["wq"][:, :, 0, 0].T.reshape(2, 128, 32).astype(bf)
    projw = np.zeros((128, 640), bf)
    for ch in range(2):
        projw[:, 320 * ch:320 * ch + 256] = wvTg[ch]
        projw[:, 320 * ch + 256:320 * ch + 288] = wkT[ch]
        projw[:, 320 * ch + 288:320 * ch + 320] = wqT[ch]

    pwall = np.zeros((128, 2048), np.float32)
    for cb, pwT in ((0, pw1T), (1, pw2T)):
        pwall[:, 1024 * cb:1024 * (cb + 1)] = \
            pwT.astype(np.float32).transpose(1, 0, 2).reshape(128, 1024) * 64.0
    pwall = pwall.astype(ml_dtypes.float8_e4m3fn)

    def bn_fold(g, b_, mean, var, pw, b_dw, b_pw):
        s = g / np.sqrt(var + EPS) / 1024.0
        bc = pw @ b_dw + b_pw
        t = s * (bc - mean) + b_
        o = np.zeros((128, 4), f)
        o[:, 0], o[:, 1] = s[0:128], t[0:128]
        o[:, 2], o[:, 3] = s[128:256], t[128:256]
        return o

    consts = np.zeros((128, 11), f)
    consts[:, 0:4] = bn_fold(inputs["bn1_g"], inputs["bn1_b"], inputs["bn1_m"],
                             inputs["bn1_v"], pw1, inputs["b1_dw"],
                             inputs["b1_pw"])
    consts[:, 4:8] = bn_fold(inputs["bn2_g"], inputs["bn2_b"], inputs["bn2_m"],
                             inputs["bn2_v"], pw2, inputs["b2_dw"],
                             inputs["b2_pw"])
    consts[0:32, 8] = inputs["bk"]
    consts[32:64, 8] = inputs["bq"]
    consts[:, 9] = gamma * inputs["bv"][0:128]
    consts[:, 10] = gamma * inputs["bv"][128:256]

    return dict(w1bd=w1bd, w2bd=w2bd,
                projw=projw, pwall=pwall, consts=consts)


def _prep_core(inputs, shared, b, h):
    bf = ml_dtypes.bfloat16
    x1 = inputs["x1"][b]          # [256, 64, 64]
    x2 = inputs["x2"][b]
    sub = x1 - x2
    cat1 = np.concatenate([sub, x1], axis=0).reshape(4, 128, 64, 64)
    cat2 = np.concatenate([sub, x2], axis=0).reshape(4, 128, 64, 64)

    f8 = ml_dtypes.float8_e4m3fn

    def pack(cc, wbd):
        buf = np.zeros((4, 128, SLOTS, PW8), np.float32)
        if h == 0:
            buf[:, :, 1:34, 1:65] = cc[:, :, 0:33, :]
        else:
            buf[:, :, 0:33, 1:65] = cc[:, :, 31:64, :]
        flat = buf.reshape(4, 128, CAT80).astype(f8)
        cw = np.zeros((4, 128, CATW), f8)
        cw[:, :, 0:CAT80] = flat
        cw[:, :, AREG:AREG + CAT80 - 1] = flat[:, :, 1:]
        cw[:, :, 2 * AREG:] = wbd
        return cw

    x1r = x1.reshape(256, N)[:, QH * h:QH * (h + 1)]   # [256, QH]
    x1h = np.ascontiguousarray(
        np.concatenate([x1r[0:128], x1r[128:256]], axis=1))  # [128, 2*QH]
    return dict(c1w=pack(cat1, shared["w1bd"]),
                c2w=pack(cat2, shared["w2bd"]),
                x1h=x1h)


def kernel(**inputs):
    if "nc" not in _CACHE:
        _CACHE["nc"] = _build_nc()
    nc = _CACHE["nc"]

    inputs = {k: np.ascontiguousarray(np.asarray(v)) for k, v in inputs.items()}
    shared = _prep_shared(inputs)
    in_maps = []
    for core in range(8):
        b, h = core // 2, core % 2
        m = dict(projw=shared["projw"], pwall=shared["pwall"],
                 consts=shared["consts"])
        m.update(_prep_core(inputs, shared, b, h))
        in_maps.append(m)

    res = run_bass_kernel_spmd(nc, in_maps, list(range(8)))
    out = np.empty((4, 256, N), np.float32)
    for core in range(8):
        b, h = core // 2, core % 2
        r = res.results[core]["out"]
        out[b, 0:128, QH * h:QH * (h + 1)] = r[:, 0:QH]
        out[b, 128:256, QH * h:QH * (h + 1)] = r[:, QH:2 * QH]
    return out.reshape(B, C, H, W)


# revision 28
# speedup vs baseline: 1.2655x; 1.0192x over previous
"""CrossAttention kernel for Trainium2, 8 NeuronCores.

Reference pipeline (B=4, C=256, H=W=64, N=4096, d=C//8=32):
  sub = x1 - x2
  x3 = relu(bn1(pw1(dw1([sub, x1]))))      # dw: 3x3 grouped conv (groups=C)
  x4 = relu(bn2(pw2(dw2([sub, x2]))))      # pw: 1x1 512->256
  q = wq@x4 [B,32,N]; k = wk@x3 [B,32,N]; v = wv@x3 [B,256,N]
  attn = softmax(q^T k);  out = gamma * (v @ attn^T) + x1

Sharding: 8 cores = (batch b) x (pixel-half h). Each core computes BOTH
conv paths only for its own pixel half (with a one-row halo), projects
k / v^T / q from its half, then the pair exchanges k and v^T via a
single AllGather so each core runs flash attention for its 2048 queries
over all 4096 keys.

Device-side choices:
  - dw conv on the PE as 9 accumulating block-diagonal [128x128] matmuls
    over a zero-padded 66-col image layout (8-row windows = 512 output
    columns via a strided rhs AP that skips the pad columns).
  - energy is computed transposed, E^T[j, i], with k/q zero-padded to a
    128-deep contraction so the PE streams at full rate.
  - attention accumulation outputs [channel, query] directly (lhsT =
    v^T block, rhs = exp tile): 512-wide matmuls, no PE transposes. The
    softmax denominator comes from an all-ones lhsT matmul which also
    broadcasts it across partitions for the DVE normalize.
  - all inputs packed into few large DMAs (descriptor generation on the
    sync engine costs ~0.7us per dma_start, so count matters).
  - matmul datapath in bf16 (fp32 PSUM accumulation); normalize /
    residual-add in fp32. gamma folded into wv/bv on the host; bn+biases
    folded into per-channel scale/shift applied during PSUM eviction.
"""

import numpy as np
import ml_dtypes

import concourse.bass as bass
import concourse.mybir as mybir
import concourse.tile as tile
from concourse import bacc
from concourse.bass_utils import run_bass_kernel_spmd

F32 = mybir.dt.float32
BF16 = mybir.dt.bfloat16
F8 = mybir.dt.float8e4
U8 = mybir.dt.uint8
PM = mybir.MatmulPerfMode
AF = mybir.ActivationFunctionType
ALU = mybir.AluOpType

B, C, H, W = 4, 256, 64, 64
N = H * W            # 4096 pixels
QH = N // 2          # pixels per core (queries/own keys)
EPS = 1e-5
PW = 66              # padded row width
OFF = 2              # leading pad elements in padded tiles
SLOTS = 34           # 32 data rows + halo/zero rows
CAT_F = OFF + SLOTS * PW + OFF   # 2248 (legacy bf16 layout, unused)
PW8 = 80                         # fp8 padded row stride (16B-aligned)
CAT80 = SLOTS * PW8              # 2720 flat image bytes
AREG = 2736                      # per-copy region (16B-aligned)
CATW = 2 * AREG + 9 * 128        # image copy A | shifted copy B | dw weights
VT = 272             # v^T row stride: 256 channels + pad (16B-aligned for DoubleRow)
PAIRS = [[0, 1], [2, 3], [4, 5], [6, 7]]

_CACHE = {}


def _build_nc():
    nc = bacc.Bacc("TRN2", target_bir_lowering=False, debug=False, num_devices=8)

    c1w_d = nc.dram_tensor("c1w", [4, 128, CATW], F8, kind="ExternalInput")
    c2w_d = nc.dram_tensor("c2w", [4, 128, CATW], F8, kind="ExternalInput")
    x1h_d = nc.dram_tensor("x1h", [128, 2 * QH], F32, kind="ExternalInput")
    consts_d = nc.dram_tensor("consts", [128, 11], F32, kind="ExternalInput")
    projw_d = nc.dram_tensor("projw", [128, 640], BF16, kind="ExternalInput")
    pwall_d = nc.dram_tensor("pwall", [128, 2048], F8, kind="ExternalInput")
    out_d = nc.dram_tensor("out", [128, 2 * QH], F32, kind="ExternalOutput")

    # collective bounce buffers, in bytes: v^T fp8 (128x4128) + k bf16 (32x2048)
    VSZ = 128 * 16 * VT              # v^T bytes
    KVN = VSZ + 32 * QH * 2          # + k bytes = 659456
    kvown_d = nc.dram_tensor("kvown_b", [KVN], U8)
    kvfull_d = nc.dram_tensor("kvfull_b", [2 * KVN], U8)

    with tile.TileContext(nc) as tc:
        with tc.tile_pool(name="persist", bufs=1) as pp:
            catw = [[pp.tile([128, CATW], F8, name=f"cw{b}_{k}",
                             tag=f"cw{b}_{k}") for k in range(4)]
                    for b in range(2)]
            x3o = [pp.tile([128, QH], BF16, name=f"x3o_{m}", tag=f"x3o_{m}")
                   for m in range(2)]
            x4 = [pp.tile([128, QH], BF16, name=f"x4_{m}", tag=f"x4_{m}")
                  for m in range(2)]
            consts = pp.tile([128, 11], F32, name="consts", tag="consts")
            projw = pp.tile([128, 640], BF16, name="projw", tag="projw")
            pwall = pp.tile([128, 2048], F8, name="pwall", tag="pwall")
            k_own = pp.tile([128, QH], BF16, name="k_own", tag="k_own")
            vto = pp.tile([128, 16 * VT], F8, name="vto", tag="vto")
            k_sb = pp.tile([128, N], BF16, name="k_sb", tag="k_sb")
            q_sb = pp.tile([128, QH], BF16, name="q_sb", tag="q_sb")
            vta = pp.tile([128, 32 * VT], F8, name="vta", tag="vta")
            x1h = pp.tile([128, 2 * QH], F32, name="x1h", tag="x1h")
            out_sb = pp.tile([128, 2 * QH], F32, name="osb", tag="osb")
            ones_sb = pp.tile([128, 256], F8, name="ones", tag="ones")
            # zero rows 32:128 so energy matmuls can use full 128-row lhsT
            # (avoids the PE small-tile row-group slowdown)
            nc.gpsimd.memset(k_sb[:], 0.0)
            nc.gpsimd.memset(q_sb[:], 0.0)
            nc.gpsimd.memset(ones_sb[:], 1.0)

            # input DMAs in priority order (conv1, consts, conv2, residual);
            # first pack split across partition halves so conv1 starts sooner
            nc.sync.dma_start(catw[0][0][0:64, :], c1w_d[0][0:64])
            nc.sync.dma_start(catw[0][0][64:128, :], c1w_d[0][64:128])
            for k in range(1, 4):
                nc.sync.dma_start(catw[0][k][:], c1w_d[k])
            nc.sync.dma_start(consts[:], consts_d[:])
            nc.sync.dma_start(pwall[:], pwall_d[:])
            nc.sync.dma_start(projw[:], projw_d[:])
            for k in range(4):
                nc.sync.dma_start(catw[1][k][:], c2w_d[k])
            nc.sync.dma_start(x1h[:], x1h_d[:])

            pwr = pwall[:].rearrange("p (b k m) -> p b k m", b=2, k=4)

            def conv_block(cb, cw, bno, xout, after_window=None):
                # dw in bf16 (shifted-window APs rule out DoubleRow); y1
                # evicted x16 into fp8 so the 1x1 pw conv runs as fp8
                # DoubleRow k-tile pairs (weights x64; bn scale /1024)
                WO = 2 * AREG
                with tc.tile_pool(name="conv_y", bufs=2) as cyb, \
                     tc.tile_pool(name="conv_ps", bufs=2, space="PSUM") as cps:
                    for w in range(4):
                        y1w = cyb.tile([128, 2048], F8, name="y1w", tag="y1w")
                        y1r = y1w[:].rearrange("p (k f) -> p k f", k=4)
                        for k in range(4):
                            ab = cw[k][:, 0:WO].rearrange("p (c f) -> p c f",
                                                          c=2)
                            for sub in range(2):
                                w4 = 2 * w + sub
                                ps = cps.tile([128, 320], F32, name="dwps",
                                              tag="dwps")
                                for dr in range(3):
                                    s = (4 * w4 + dr) * PW8
                                    nc.tensor.matmul(
                                        ps[:],
                                        cw[k][:, WO + 384 * dr:
                                              WO + 384 * dr + 256]
                                        .rearrange("p (c f) -> p c f", c=2),
                                        ab[:, :, s:s + 320],
                                        start=(dr == 0), stop=False,
                                        perf_mode=PM.DoubleRow)
                                    nc.tensor.matmul(
                                        ps[:],
                                        cw[k][:, WO + 384 * dr + 256:
                                              WO + 384 * dr + 384],
                                        cw[k][:, s + 2:s + 322],
                                        start=False, stop=(dr == 2))
                                nc.scalar.activation(
                                    y1w[:, 512 * k + 256 * sub:
                                        512 * k + 256 * (sub + 1)],
                                    ps[:, 0:320].rearrange(
                                        "p (r c) -> p r c", r=4,
                                        c=PW8)[:, :, 0:64],
                                    AF.Copy, scale=0.25)
                        for m in range(2):
                            pp2 = cps.tile([128, 512], F32, name="pwps", tag="pwps")
                            for kp in range(2):
                                nc.tensor.matmul(
                                    pp2[:],
                                    pwr[:, cb, 2 * kp:2 * kp + 2,
                                        128 * m:128 * (m + 1)],
                                    y1r[:, 2 * kp:2 * kp + 2, :],
                                    start=(kp == 0), stop=(kp == 1),
                                    perf_mode=PM.DoubleRow)
                            nc.scalar.activation(
                                xout[m][:, 512 * w:512 * (w + 1)], pp2[:],
                                AF.Relu,
                                bias=consts[:, bno + 2 * m + 1:bno + 2 * m + 2],
                                scale=consts[:, bno + 2 * m:bno + 2 * m + 1])
                        if after_window is not None:
                            after_window(w)

            # projections interleaved into conv1's window loop: k / v^T
            # blocks only need that window's x3 columns, so the pair
            # exchange can trigger right at conv1's end
            pps_c = tc.tile_pool(name="proj_ps", bufs=2, space="PSUM")
            pps = pps_c.__enter__()

            def proj1_window(s):
                ps = pps.tile([128, 512], F32, name="kqps", tag="kqps")
                for ch in range(2):
                    nc.tensor.matmul(ps[0:32, :],
                                     projw[:, 320 * ch + 256:320 * ch + 288],
                                     x3o[ch][:, 512 * s:512 * (s + 1)],
                                     start=(ch == 0), stop=(ch == 1))
                nc.scalar.activation(k_own[0:32, 512 * s:512 * (s + 1)],
                                     ps[0:32, :], AF.Identity,
                                     bias=consts[0:32, 8:9])
                for jp in range(2 * s, 2 * s + 2):
                    ps = pps.tile([128, 512], F32, name="vtps", tag="vtps")
                    for u in range(2):
                        for ch in range(2):
                            nc.tensor.matmul(
                                ps[:, 256 * u:256 * (u + 1)],
                                x3o[ch][:, 128 * (2 * jp + u):
                                        128 * (2 * jp + u + 1)],
                                projw[:, 320 * ch:320 * ch + 256],
                                start=(ch == 0), stop=(ch == 1))
                    nc.scalar.activation(
                        vto[:].rearrange("p (j v) -> p j v",
                                         j=16)[:, 2 * jp:2 * jp + 2, 0:256],
                        ps[:].rearrange("p (u v) -> p u v", u=2),
                        AF.Copy)

            conv_block(0, catw[0], 0, x3o, after_window=proj1_window)

            if True:
                # ship own k / v^T, single AllGather for the pair
                nc.sync.dma_start(
                    kvown_d[0:VSZ].rearrange("(p f) -> p f", p=128),
                    vto[:].bitcast(U8))
                nc.sync.dma_start(
                    kvown_d[VSZ:KVN].rearrange("(p f) -> p f", p=32),
                    k_own[0:32, :].bitcast(U8))
                nc.gpsimd.collective_compute(
                    "AllGather", ALU.bypass, replica_groups=PAIRS,
                    ins=[kvown_d[:].opt()], outs=[kvfull_d[:].opt()])
                for m in range(2):
                    o = m * KVN
                    nc.sync.dma_start(
                        vta[:, 16 * VT * m:16 * VT * (m + 1)].bitcast(U8),
                        kvfull_d[o:o + VSZ].rearrange("(p f) -> p f", p=128))
                    nc.sync.dma_start(
                        k_sb[0:32, QH * m:QH * (m + 1)].bitcast(U8),
                        kvfull_d[o + VSZ:o + KVN].rearrange("(p f) -> p f",
                                                            p=32))

                # conv2 + q overlap with the collective
                def proj2_window(s):
                    ps = pps.tile([128, 512], F32, name="kqps", tag="kqps")
                    for ch in range(2):
                        nc.tensor.matmul(ps[0:32, :],
                                         projw[:, 320 * ch + 288:320 * ch + 320],
                                         x4[ch][:, 512 * s:512 * (s + 1)],
                                         start=(ch == 0), stop=(ch == 1))
                    nc.scalar.activation(q_sb[0:32, 512 * s:512 * (s + 1)],
                                         ps[0:32, :], AF.Identity,
                                         bias=consts[32:64, 8:9])

                conv_block(1, catw[1], 4, x4, after_window=proj2_window)
            pps_c.__exit__(None, None, None)

            # ---- flash attention (output computed pre-transposed) ----
            # single flattened pipeline over 64 key-block pairs (4 query
            # blocks x 16) so the PE/scalar pipeline never drains at query
            # block boundaries; normalize uses a fused divide stt so acc
            # banks free quickly
            with tc.tile_pool(name="att_sb", bufs=3) as asb, \
                 tc.tile_pool(name="acc_ps", bufs=1, space="PSUM") as accp, \
                 tc.tile_pool(name="dps_ps", bufs=2, space="PSUM") as dpsp, \
                 tc.tile_pool(name="e_ps", bufs=2, space="PSUM") as epsp:
                vv = vta[:].rearrange("p (j v) -> p j v", j=32)
                ones2 = ones_sb[:].rearrange("p (k f) -> p k f", k=2)
                eps_t = {}
                acc = None
                dps = None

                def energy_pair(g):
                    ib = g // 16
                    p = g % 16
                    eps_t[g] = epsp.tile([128, 1024], F32, name="eps",
                                         tag="eps")
                    for h in range(2):
                        nc.tensor.matmul(
                            eps_t[g][:, 512 * h:512 * (h + 1)],
                            k_sb[:, 128 * (2 * p + h):128 * (2 * p + h + 1)],
                            q_sb[:, 512 * ib:512 * (ib + 1)],
                            start=True, stop=True)

                for g in range(64):
                    ib, p = g // 16, g % 16
                    if g == 0:
                        energy_pair(0)
                    if p == 0:
                        acc = [accp.tile([128, 512], F32, name=f"acc{c}",
                                         tag=f"acc{c}") for c in range(2)]
                        dps = dpsp.tile([128, 512], F32, name="dps", tag="dps")
                    expair = asb.tile([128, 1024], F8, name="ex", tag="ex")
                    nc.scalar.activation(expair[:], eps_t[g][:], AF.Exp)
                    if g + 1 < 64:
                        energy_pair(g + 1)
                    eps_t.pop(g - 1, None)
                    rhs2 = expair[:].rearrange("p (k f) -> p k f", k=2)
                    nc.tensor.matmul(acc[0][:], vv[:, 2 * p:2 * p + 2, 0:128],
                                     rhs2, start=(p == 0), stop=(p == 15),
                                     perf_mode=PM.DoubleRow)
                    nc.tensor.matmul(acc[1][:],
                                     vv[:, 2 * p:2 * p + 2, 128:256],
                                     rhs2, start=(p == 0), stop=(p == 15),
                                     perf_mode=PM.DoubleRow)
                    nc.tensor.matmul(dps[:], ones2, rhs2,
                                     start=(p == 0), stop=(p == 15),
                                     perf_mode=PM.DoubleRow)
                    if p == 15:
                        # normalize: PSUM-reading ops first so acc banks
                        # free before the next query block needs them
                        rec = asb.tile([128, 512], F32, name="rec", tag="rec")
                        nc.vector.reciprocal_approx_fast(rec[:], dps[:])
                        tmp = [asb.tile([128, 512], BF16, name=f"tmp{c}",
                                        tag=f"tmp{c}") for c in range(2)]
                        for ch in range(2):
                            nc.vector.scalar_tensor_tensor(
                                tmp[ch][:], acc[ch][:], 1.0, rec[:],
                                ALU.mult, ALU.mult)
                        for ch in range(2):
                            nc.vector.scalar_tensor_tensor(
                                out_sb[:,
                                       QH * ch + 512 * ib:QH * ch + 512 * (ib + 1)],
                                tmp[ch][:], consts[:, 9 + ch:10 + ch],
                                x1h[:,
                                    QH * ch + 512 * ib:QH * ch + 512 * (ib + 1)],
                                ALU.add, ALU.add)
                        if ib < 3:
                            nc.sync.dma_start(
                                out_d[:].rearrange(
                                    "p (c f) -> p c f",
                                    c=2)[:, :, 512 * ib:512 * (ib + 1)],
                                out_sb[:].rearrange(
                                    "p (c f) -> p c f",
                                    c=2)[:, :, 512 * ib:512 * (ib + 1)])
                        else:
                            # last block: per-channel halves so the first DMA
                            # overlaps the second half's normalize
                            for ch in range(2):
                                o = QH * ch + 512 * ib
                                nc.sync.dma_start(out_d[:, o:o + 512],
                                                  out_sb[:, o:o + 512])
    nc.compile()
    return nc


def _prep_shared(inputs):
    f = np.float32
    bf = ml_dtypes.bfloat16

    def bd(w_dw):
        wr = w_dw.reshape(512, 2, 9)
        Wt = np.zeros((4, 128, 9, 128), f)
        m = np.arange(64)
        for k in range(4):
            blk = wr[128 * k:128 * (k + 1)]        # [128, 2, 9]
            for i in range(2):
                for j in range(2):
                    Wt[k, 2 * m + i, :, 2 * m + j] = blk[2 * m + j, i, :]
        return np.ascontiguousarray(
            Wt.reshape(4, 128, 9 * 128) * 64.0).astype(
                ml_dtypes.float8_e4m3fn)

    w1bd = bd(inputs["w1_dw"])
    w2bd = bd(inputs["w2_dw"])

    pw1 = inputs["w1_pw"][:, :, 0, 0]              # [256, 512]
    pw2 = inputs["w2_pw"][:, :, 0, 0]
    pw1T = np.ascontiguousarray(pw1.T.reshape(4, 128, 256)).astype(bf)
    pw2T = np.ascontiguousarray(pw2.T.reshape(4, 128, 256)).astype(bf)

    gamma = float(inputs["gamma"][0])
    wvTg = (inputs["wv"][:, :, 0, 0].T * gamma).reshape(2, 128, 256).astype(bf)
    wkT = inputs["wk"][:, :, 0, 0].T.reshape(2, 128, 32).astype(bf)
    wqT = inputs["wq"][:, :, 0, 0].T.reshape(2, 128, 32).astype(bf)
    projw = np.zeros((128, 640), bf)
    for ch in range(2):
        projw[:, 320 * ch:320 * ch + 256] = wvTg[ch]
        projw[:, 320 * ch + 256:320 * ch + 288] = wkT[ch]
        projw[:, 320 * ch + 288:320 * ch + 320] = wqT[ch]

    pwall = np.zeros((128, 2048), np.float32)
    for cb, pwT in ((0, pw1T), (1, pw2T)):
        pwall[:, 1024 * cb:1024 * (cb + 1)] = \
            pwT.astype(np.float32).transpose(1, 0, 2).reshape(128, 1024) * 64.0
    pwall = pwall.astype(ml_dtypes.float8_e4m3fn)

    def bn_fold(g, b_, mean, var, pw, b_dw, b_pw):
        s = g / np.sqrt(var + EPS) / 1024.0
        bc = pw @ b_dw + b_pw
        t = s * (bc - mean) + b_
        o = np.zeros((128, 4), f)
        o[:, 0], o[:, 1] = s[0:128], t[0:128]
        o[:, 2], o[:, 3] = s[128:256], t[128:256]
        return o

    consts = np.zeros((128, 11), f)
    consts[:, 0:4] = bn_fold(inputs["bn1_g"], inputs["bn1_b"], inputs["bn1_m"],
                             inputs["bn1_v"], pw1, inputs["b1_dw"],
                             inputs["b1_pw"])
    consts[:, 4:8] = bn_fold(inputs["bn2_g"], inputs["bn2_b"], inputs["bn2_m"],
                             inputs["bn2_v"], pw2, inputs["b2_dw"],
                             inputs["b2_pw"])
    consts[0:32, 8] = inputs["bk"]
    consts[32:64, 8] = inputs["bq"]
    consts[:, 9] = gamma * inputs["bv"][0:128]
    consts[:, 10] = gamma * inputs["bv"][128:256]

    return dict(w1bd=w1bd, w2bd=w2bd,
                projw=projw, pwall=pwall, consts=consts)


def _prep_core(inputs, shared, b, h):
    bf = ml_dtypes.bfloat16
    x1 = inputs["x1"][b]          # [256, 64, 64]
    x2 = inputs["x2"][b]
    sub = x1 - x2
    cat1 = np.concatenate([sub, x1], axis=0).reshape(4, 128, 64, 64)
    cat2 = np.concatenate([sub, x2], axis=0).reshape(4, 128, 64, 64)

    f8 = ml_dtypes.float8_e4m3fn

    def pack(cc, wbd):
        buf = np.zeros((4, 128, SLOTS, PW8), np.float32)
        if h == 0:
            buf[:, :, 1:34, 1:65] = cc[:, :, 0:33, :]
        else:
            buf[:, :, 0:33, 1:65] = cc[:, :, 31:64, :]
        flat = buf.reshape(4, 128, CAT80).astype(f8)
        cw = np.zeros((4, 128, CATW), f8)
        cw[:, :, 0:CAT80] = flat
        cw[:, :, AREG:AREG + CAT80 - 1] = flat[:, :, 1:]
        cw[:, :, 2 * AREG:] = wbd
        return cw

    x1r = x1.reshape(256, N)[:, QH * h:QH * (h + 1)]   # [256, QH]
    x1h = np.ascontiguousarray(
        np.concatenate([x1r[0:128], x1r[128:256]], axis=1))  # [128, 2*QH]
    return dict(c1w=pack(cat1, shared["w1bd"]),
                c2w=pack(cat2, shared["w2bd"]),
                x1h=x1h)


def kernel(**inputs):
    if "nc" not in _CACHE:
        _CACHE["nc"] = _build_nc()
    nc = _CACHE["nc"]

    inputs = {k: np.ascontiguousarray(np.asarray(v)) for k, v in inputs.items()}
    shared = _prep_shared(inputs)
    in_maps = []
    for core in range(8):
        b, h = core // 2, core % 2
        m = dict(projw=shared["projw"], pwall=shared["pwall"],
                 consts=shared["consts"])
        m.update(_prep_core(inputs, shared, b, h))
        in_maps.append(m)

    res = run_bass_kernel_spmd(nc, in_maps, list(range(8)))
    out = np.empty((4, 256, N), np.float32)
    for core in range(8):
        b, h = core // 2, core % 2
        r = res.results[core]["out"]
        out[b, 0:128, QH * h:QH * (h + 1)] = r[:, 0:QH]
        out[b, 128:256, QH * h:QH * (h + 1)] = r[:, QH:2 * QH]
    return out.reshape(B, C, H, W)
